# revision 3
# baseline (speedup 1.0000x reference)
"""Masked-BCE valid-region loss on 8 Trainium2 NeuronCores.

Inputs (full): cancer_logits [32,1,512,512] f32, label [32] f32,
prostate_mask [32,1,512,512] f32, needle_mask [32,1,512,512] f32.
Output: scalar f32 loss.

Sharding: data-parallel over batch - 4 images per core. The host packs
all three tensors as fp8e4m3 (1 byte/pixel; the 2e-2 harness tolerance
dwarfs the rounding effect, and mask threshold flips hit numerator and
denominator on the same pixels so the ratio moves ~1e-4): 3 MB of HBM
traffic per core vs 12 MB in f32.

Math: with m = (min(p,n) > 0.5) and y constant per image,

    bce = softplus(x) - x*y,   softplus(z) = -ln(sigmoid(-z))
    sum_masked softplus(x) = -sum ln(sigmoid(-xm)) - (N - count)*ln2

since masked-out elements have xm = 0 and contribute sigmoid(0) = 1/2.
The ln of 1M products is folded as ln(prod) over groups of 8 via a
bf16 multiply tree (sigma <= 1 so products only shrink; worst case
8*6.55 = 52 lns stays far above bf16 underflow), so the ACT engine
runs ONE full-size pass (sigmoid) instead of exp+ln, and the final ln
touches N/8 elements.

Device pipeline per chunk:

    pb,nb,xb = cast-DMA fp8->bf16      # SWDGE casts during the DMA
    pb  = min(pb, nb)                  # DVE tensor_tensor, 2x at bf16
    m   = (pb > 0.5)                   # DVE tensor_scalar, 4x
    xm  = m * xb                       # DVE tensor_tensor, 2x
    s   = sigmoid(-xm)                 # ACT (the only full-size pass)
    s2/s4/s8 = halve-multiply tree     # DVE 2x, contiguous halves
    cnt += ones' @ m;  sxm_img += ones' @ xm   # TensorE -> PSUM
    ...all chunks done...
    ln(s8) with accum_out per chunk    # ACT, one table switch total

tensor_scalar cannot carry an accumulator (BIR verifier rejects it)
and Pool/GpSimd cannot run TensorScalarPtr at all, so count and the
per-image sum(x*m) ride TensorE ones-matmuls into PSUM banks. The act
tables are pinned to {sigmoid_and_others, natural_log} so exactly two
ACT_TABLE_LOADs are emitted.
"""

import sys

for _p in ("/opt/trn_rl_repo", "/root/.axon_site/_ro/trn_rl_repo"):
    if _p not in sys.path:
        sys.path.append(_p)

import ml_dtypes
import numpy as np

import concourse.bacc as bacc
import concourse.tile as tile
from concourse import mybir
from concourse.bass_utils import run_bass_kernel_spmd

B, H, W = 32, 512, 512
N_CORES = 8
IMGS_PER_CORE = B // N_CORES  # 4
P = 128
FD = (H * W) // P  # 2048 free-dim elements per partition per image
N_PER_IMG = H * W  # 262144
TOT_FD = IMGS_PER_CORE * FD  # 8192
# image-aligned chunks (multiples of 512 for PE blocks, of 8 for the tree);
# smaller edge chunks shorten pipeline ramp and tail.
CHUNK_FDS = [512, 1536, 2048, 2048, 1536, 512]
N_CHUNKS = len(CHUNK_FDS)
TREE_K = 8  # elements per ln group; 8*6.55 < 88 so bf16 never underflows

_nc_cache = None


def _patch_act_tables():
    """Keep only {sigmoid_and_others, natural_log} activation sets so the
    per-activation table picker emits exactly one ACT_TABLE_LOAD per set
    (sigmoid for the main pass, ln for the batched tail) instead of
    reloading alternating sets."""
    import concourse.hw_specs as hw_specs

    if getattr(bacc, "_act_tables_patched", False):
        return
    orig = hw_specs.get_activation_tables

    def patched(module_arch):
        tables = orig(module_arch)
        keep = ("sigmoid_and_others", "natural_log")
        return {
            name: (funcs if name in keep else set())
            for name, funcs in tables.items()
        }

    bacc.get_activation_tables = patched
    bacc._act_tables_patched = True


def _build_bass():
    _patch_act_tables()
    f32 = mybir.dt.float32
    bf16 = mybir.dt.bfloat16
    fp8 = mybir.dt.float8e4
    nc = bacc.Bacc()
    p_d = nc.dram_tensor("p8", [P, TOT_FD], fp8, kind="ExternalInput")
    n_d = nc.dram_tensor("n8", [P, TOT_FD], fp8, kind="ExternalInput")
    x_d = nc.dram_tensor("x8", [P, TOT_FD], fp8, kind="ExternalInput")
    # ln-accum columns, one per chunk
    ln_o = nc.dram_tensor("lncols", [P, N_CHUNKS], f32, kind="ExternalOutput")
    # [count row | img0..img3 rows], 512 f32 each
    red_o = nc.dram_tensor("red", [1, 5 * 512], f32, kind="ExternalOutput")

    with tile.TileContext(nc) as tc:
        with (
            tc.tile_pool(name="io", bufs=2) as io_pool,
            tc.tile_pool(name="work", bufs=2) as work_pool,
            tc.tile_pool(name="keep", bufs=1) as keep_pool,
            tc.tile_pool(name="psum", bufs=1, space="PSUM") as psum_pool,
        ):
            ones1 = keep_pool.tile([P, 1], bf16)
            nc.vector.memset(ones1, 1.0)
            lncols = keep_pool.tile([P, N_CHUNKS], f32)
            cnt_ps = psum_pool.tile([1, 512], f32, tag="cnt")
            img_ps = [
                psum_pool.tile([1, 512], f32, tag=f"img{i}", name=f"img_ps{i}")
                for i in range(IMGS_PER_CORE)
            ]

            total_blocks = TOT_FD // 512  # count matmuls over all chunks
            blocks_per_img = FD // 512  # per-image xm matmuls
            cnt_done = 0
            img_done = [0] * IMGS_PER_CORE
            s8_tiles = []
            off = 0
            for c, cfd in enumerate(CHUNK_FDS):
                pb = io_pool.tile([P, cfd], bf16, tag="pb")
                nc.gpsimd.dma_start(out=pb, in_=p_d[:, off : off + cfd])
                nb = io_pool.tile([P, cfd], bf16, tag="nb")
                nc.gpsimd.dma_start(out=nb, in_=n_d[:, off : off + cfd])
                xb = io_pool.tile([P, cfd], bf16, tag="xb")
                nc.gpsimd.dma_start(out=xb, in_=x_d[:, off : off + cfd])

                # pt = min(p, n), in place over pb
                nc.vector.tensor_tensor(
                    out=pb, in0=pb, in1=nb, op=mybir.AluOpType.min
                )
                m = work_pool.tile([P, cfd], bf16, tag="m")
                nc.vector.tensor_scalar(
                    out=m, in0=pb, scalar1=0.5, scalar2=None,
                    op0=mybir.AluOpType.is_gt,
                )
                xm = work_pool.tile([P, cfd], bf16, tag="xm")
                nc.vector.tensor_tensor(
                    out=xm, in0=m, in1=xb, op=mybir.AluOpType.mult
                )
                # s = sigmoid(-xm); bf16 out feeds the 2x multiply tree
                s = work_pool.tile([P, cfd], bf16, tag="s")
                nc.scalar.activation(
                    out=s, in_=xm, func=mybir.ActivationFunctionType.Sigmoid,
                    scale=-1.0,
                )
                h = cfd // 2
                s2 = work_pool.tile([P, h], bf16, tag="s2")
                nc.vector.tensor_tensor(
                    out=s2, in0=s[:, :h], in1=s[:, h:], op=mybir.AluOpType.mult
                )
                q = cfd // 4
                s4 = work_pool.tile([P, q], bf16, tag="s4")
                nc.vector.tensor_tensor(
                    out=s4, in0=s2[:, :q], in1=s2[:, q:], op=mybir.AluOpType.mult
                )
                e = cfd // 8
                s8 = keep_pool.tile([P, e], bf16, tag=f"s8_{c}")
                nc.vector.tensor_tensor(
                    out=s8, in0=s4[:, :e], in1=s4[:, e:], op=mybir.AluOpType.mult
                )
                s8_tiles.append((s8, c))

                # TensorE reductions: count (bank cnt) + per-image sum(xm)
                img = off // FD
                for s0 in range(0, cfd, 512):
                    nc.tensor.matmul(
                        cnt_ps, ones1, m[:, s0 : s0 + 512],
                        start=(cnt_done == 0),
                        stop=(cnt_done == total_blocks - 1),
                    )
                    cnt_done += 1
                    i = (off + s0) // FD
                    nc.tensor.matmul(
                        img_ps[i], ones1, xm[:, s0 : s0 + 512],
                        start=(img_done[i] == 0),
                        stop=(img_done[i] == blocks_per_img - 1),
                    )
                    img_done[i] += 1
                off += cfd

            # batched ln tail: one table switch, accum per chunk
            for s8, c in s8_tiles:
                lnv = work_pool.tile([P, s8.shape[1]], bf16, tag="lnv")
                nc.scalar.activation(
                    out=lnv, in_=s8, func=mybir.ActivationFunctionType.Ln,
                    accum_out=lncols[:, c : c + 1],
                )

            red_sb = keep_pool.tile([1, 5 * 512], f32)
            nc.vector.tensor_scalar_add(out=red_sb[:, 0:512], in0=cnt_ps, scalar1=0.0)
            for i in range(IMGS_PER_CORE):
                nc.vector.tensor_scalar_add(
                    out=red_sb[:, (i + 1) * 512 : (i + 2) * 512],
                    in0=img_ps[i], scalar1=0.0,
                )
            nc.sync.dma_start(out=ln_o[:], in_=lncols)
            nc.sync.dma_start(out=red_o[:], in_=red_sb)
    nc.finalize()
    return nc


def _get_nc():
    global _nc_cache
    if _nc_cache is None:
        _nc_cache = _build_bass()
    return _nc_cache


def _make_in_maps(cancer_logits, prostate_mask, needle_mask):
    f8 = ml_dtypes.float8_e4m3
    # [B,1,H,W] -> [CORE, P, IMG*FD] image-major flat per-partition streams
    def pack(a):
        a = np.asarray(a, dtype=np.float32).reshape(B, P, FD).astype(f8)
        a = a.reshape(N_CORES, IMGS_PER_CORE, P, FD).transpose(0, 2, 1, 3)
        return np.ascontiguousarray(a).reshape(N_CORES, P, TOT_FD)

    x8 = pack(cancer_logits)
    p8 = pack(prostate_mask)
    n8 = pack(needle_mask)
    return [
        {"p8": p8[c], "n8": n8[c], "x8": x8[c]} for c in range(N_CORES)
    ]


def _combine(results, label):
    y = np.asarray(label, dtype=np.float64).reshape(B)
    ln2 = float(np.log(2.0))
    n_core = IMGS_PER_CORE * N_PER_IMG
    num = 0.0
    cnt = 0.0
    for c in range(N_CORES):
        red = np.asarray(results[c]["red"], dtype=np.float64).reshape(5 * 512)
        count = red[:512].sum()
        sxm = red[512:].reshape(IMGS_PER_CORE, 512).sum(axis=1)
        lns = np.asarray(results[c]["lncols"], dtype=np.float64).sum()
        sp_masked = -lns - (n_core - count) * ln2
        y_i = y[c * IMGS_PER_CORE : (c + 1) * IMGS_PER_CORE]
        num += sp_masked - (y_i * sxm).sum()
        cnt += count
    return np.float32(num / max(cnt, 1.0))


def kernel(cancer_logits, label, prostate_mask, needle_mask):
    nc = _get_nc()
    in_maps = _make_in_maps(cancer_logits, prostate_mask, needle_mask)
    res = run_bass_kernel_spmd(nc, in_maps, core_ids=list(range(N_CORES)))
    return _combine(res.results, label)
